# revision 1
# baseline (speedup 1.0000x reference)
"""Trainium2 Bass kernel for nn_AttentionBlock (causal attention block).

Self-contained: takes FULL inputs (batch 32), shards batch over 8 NeuronCores
(4 samples/core, pure data parallel), runs a Bass/Tile kernel per core, and
gathers the full [32, 160, 32, 32] output.

Design notes:
- channels live on SBUF partitions in 'cb' order [ul(160), b(6), x(3)];
  host-side weight prep permutes/packs everything to match.
- concat_elu is decomposed as elu(h)+1 = relu(h) + min(exp(h), 1); the -1 is
  folded into the next layer's bias host-side.  exp on ScalarE, relu/min on
  VectorE tensor_scalar (2x mode), combine via scalar_tensor_tensor.
- elu "streams" (matmul rhs) are stored as [elu+ rows | pad to 192 | elu- rows]
  so every engine op's partition window starts at 0/32/64/96 (BIR verifier
  rule); pad rows are zeroed and their weight columns are zero.
- sigmoid(g) = 0.5 + 0.5*tanh(g/2): tanh shares the ACT spline-table set with
  exp, so the kernel never pays the ~2.7us ACT table switch.  nin2 'ga' rows
  are pre-scaled by 0.5 host-side; gate = U + ga05 where U = ga05*T.
- attention: S^T stored [k, q]; no max subtraction (|S| ~ 12); softmax row
  sums come for free from a ones-row appended to the V^T lhsT of the AV
  matmul; normalization deferred to after AV.
- matmuls run as float32r (full-rate fp32 PE mode, N<=512 chunks).
"""

import sys

sys.path.insert(0, "/opt/trn_rl_repo")

import contextlib

import numpy as np

import concourse.bacc as bacc
import concourse.mybir as mybir
from concourse.bass_utils import run_bass_kernel_spmd
from concourse.tile import TileContext

F32 = mybir.dt.float32
F32R = mybir.dt.float32r
AF = mybir.ActivationFunctionType
OP = mybir.AluOpType

N, XD, NF = 32, 3, 160
KD, VD = 16, 80
CK, CQ = 169, 166
HWP = 1024
NS = 4  # samples per core
NCORES = 8
EPS = 1e-7
PAD = 192  # elu- stream offset (32-aligned, >= any channel count here)
USE_F32R = True


DT_MM = F32R if USE_F32R else F32


def _r(ap):
    return ap


def chunks(total, step=128):
    return [(o, min(step, total - o)) for o in range(0, total, step)]


_PLIMIT = {0: 128, 32: 32, 64: 64, 96: 32}


def legal_segs(src_off, dst_off, length, src_sbuf=True):
    """Split a row-range copy into SBUF-legal pieces.

    SBUF engine operands must start at partition 0/32/64/96 and stay within
    {128,32,64,32} rows respectively (BIR verifier).  PSUM sources are exempt
    (src_sbuf=False) but still split at 128-row chunk bounds.
    Yields (src_tile, src_row, dst_tile, dst_row, L).
    """
    done = 0
    while done < length:
        s, d = src_off + done, dst_off + done
        sb, db = s % 128, d % 128
        L = min(_PLIMIT[db], 128 - db, length - done)
        if src_sbuf:
            L = min(L, _PLIMIT[sb], 128 - sb)
        else:
            L = min(L, 128 - sb)
        yield (s // 128, sb, d // 128, db, L)
        done += L


# ---------------------------------------------------------------- host prep --


def prep_weights(inp):
    """Numpy prep: permutations, stream packing, bias folds, 0.5 gate scaling.

    Streams (matmul rhs row order):
      C-stream  [361]: [elu+ of cb C (169) | pad->192 | elu- (169)]
      inner [192+C]:   [elu+ (C) | pad->192 | elu- (C)]
      att   [176]:     [elu+ (80) | pad->96 | elu- (80)]
    nin2 out stacking [192+C]: [0.5*ga (C) | pad->192 | gb (C)].
    Bias tiles are per-psum-chunk: tile[r, m] = bias[128*m + r].
    """
    p = {}
    perm_k = np.array(list(range(3, 169)) + list(range(0, 3)))
    perm_q = np.arange(166)

    def bias_chunked(bias):
        nm = (len(bias) + 127) // 128
        t = np.zeros((128, nm), np.float32)
        for m in range(nm):
            seg = bias[128 * m : 128 * (m + 1)]
            t[: len(seg), m] = seg
        return t

    def nin1_stream_w(Wi, perm):
        """Wi [out, 2C] -> [out, 361] over the C-stream rows + fold amount."""
        C = Wi.shape[1] // 2
        W1, W2 = Wi[:, :C][:, perm], Wi[:, C:][:, perm]
        out = np.zeros((Wi.shape[0], PAD + CK), np.float32)
        out[:, : W1.shape[1]] = W1
        out[:, PAD : PAD + W2.shape[1]] = W2
        return out, Wi.sum(1)

    kW, kfold = nin1_stream_w(inp["gkWi"], perm_k)
    vW, vfold = nin1_stream_w(inp["gvWi"], perm_k)
    qW, qfold = nin1_stream_w(inp["gqWi"], perm_q)
    # fused out stacking [h_k | pad->192 | h_v | pad->384 | h_q] = 550 rows
    Wab = np.zeros((550, PAD + CK), np.float32)
    Wab[0:169] = kW
    Wab[192:361] = vW
    Wab[384:550] = qW
    bab = np.zeros(550, np.float32)
    bab[0:169] = inp["gkbi"] - kfold
    bab[192:361] = inp["gvbi"] - vfold
    bab[384:550] = inp["gqbi"] - qfold
    p["wab_t"] = Wab.T  # [361, 550]
    p["bab"] = bias_chunked(bab)

    def inner_w(Wo, bo, out_perm):
        """-> lhsT [192+C, 192+C] over inner stream rows / gate-stacked cols."""
        C = Wo.shape[1] // 2
        W1, W2 = Wo[:, :C], Wo[:, C:]
        bias = bo - (W1.sum(1) + W2.sum(1))
        ga_w = np.concatenate([W1, W2], axis=1)[out_perm] * 0.5  # [C, 2C]
        gb_w = np.concatenate([W1, W2], axis=1)[C + out_perm]
        n = PAD + C
        Ws = np.zeros((n, n), np.float32)
        for rows, w_ in ((slice(0, C), ga_w), (slice(PAD, n), gb_w)):
            Ws[rows, 0:C] = w_[:, 0:C]
            Ws[rows, PAD : PAD + C] = w_[:, C : 2 * C]
        bs = np.zeros(n, np.float32)
        bs[0:C] = bias[out_perm] * 0.5
        bs[PAD:n] = bias[C + out_perm] * 0.5
        return Ws.T, bias_chunked(bs)

    p["wok_t"], p["bok"] = inner_w(inp["gkWo"], inp["gkbo"], perm_k)
    p["woq_t"], p["boq"] = inner_w(inp["gqWo"], inp["gqbo"], perm_q)
    p["wov_t"], p["bov"] = inner_w(inp["gvWo"], inp["gvbo"], perm_k)
    p["woo_t"], p["boo"] = inner_w(inp["goWo"], inp["gobo"], np.arange(NF))

    # grn_out nin1: rhs rows = [C-stream ul rows only | att stream]
    W1, W2 = inp["goWi"][:, :NF], inp["goWi"][:, NF:]
    wa = np.zeros((NF, PAD + CK), np.float32)
    wa[:, 0:NF] = W1
    wa[:, PAD : PAD + NF] = W2
    p["wo1a_t"] = wa.T  # [361, 160]
    W1s, W2s = inp["goWs"][:, :VD], inp["goWs"][:, VD:]
    wb = np.zeros((NF, 176), np.float32)
    wb[:, 0:VD] = W1s
    wb[:, 96 : 96 + VD] = W2s
    p["wo1b_t"] = wb.T  # [176, 160]
    p["bo1"] = bias_chunked(
        (inp["gobi"] + inp["gobs"] - inp["goWi"].sum(1) - inp["goWs"].sum(1)).astype(
            np.float32
        )
    )

    p["nk_t"] = inp["nkW"][:, perm_k].T
    p["nq_t"] = inp["nqW"][:, perm_q].T
    p["nv_t"] = inp["nvW"][:, perm_k].T
    nb = np.zeros((128, 3), np.float32)
    nb[:KD, 0] = inp["nkb"]
    nb[:KD, 1] = inp["nqb"]
    nb[:VD, 2] = inp["nvb"]
    p["nkqvb"] = nb

    pp = np.arange(128)[:, None]
    ff = np.arange(128)[None, :]
    p["masks"] = (ff > pp).astype(np.float32)
    p["eps_nzq"] = (EPS * (HWP - np.arange(HWP, dtype=np.float32)))[None, :]
    p["ident80"] = np.eye(80, dtype=np.float32)
    return {k: np.ascontiguousarray(v, dtype=np.float32) for k, v in p.items()}


WSHAPES = {
    "wab_t": (361, 550),
    "wok_t": (361, 361),
    "woq_t": (358, 358),
    "wov_t": (361, 361),
    "woo_t": (352, 352),
    "wo1a_t": (361, 160),
    "wo1b_t": (176, 160),
    "nk_t": (169, 16),
    "nq_t": (166, 16),
    "nv_t": (169, 80),
    "bab": (128, 5),
    "bok": (128, 3),
    "boq": (128, 3),
    "bov": (128, 3),
    "boo": (128, 3),
    "bo1": (128, 2),
    "nkqvb": (128, 3),
    "masks": (128, 128),
    "eps_nzq": (1, HWP),
    "ident80": (80, 80),
}
SMALL_W = {
    "bab", "bok", "boq", "bov", "boo", "bo1", "nkqvb", "masks", "eps_nzq", "ident80",
}


def build_nc(ns=NS):
    nc = bacc.Bacc("TRN2", target_bir_lowering=False, debug=False)

    x_d = nc.dram_tensor("x", [ns, XD, HWP], F32, kind="ExternalInput")
    ul_d = nc.dram_tensor("ul", [ns, NF, HWP], F32, kind="ExternalInput")
    b_d = nc.dram_tensor("b", [ns, 2 * XD, HWP], F32, kind="ExternalInput")
    out_d = nc.dram_tensor("out", [ns, NF, HWP], F32, kind="ExternalOutput")
    MM_W = {
        "wab_t", "wok_t", "woq_t", "wov_t", "woo_t", "wo1a_t", "wo1b_t",
        "nk_t", "nq_t", "nv_t", "ident80",
    }
    wd = {
        k: nc.dram_tensor(k, list(s), DT_MM if k in MM_W else F32, kind="ExternalInput")
        for k, s in WSHAPES.items()
    }

    with TileContext(nc) as tc, contextlib.ExitStack() as ctx:
        wp = ctx.enter_context(tc.tile_pool(name="wp", bufs=1))
        p1 = ctx.enter_context(tc.tile_pool(name="p1", bufs=1))
        p2 = ctx.enter_context(tc.tile_pool(name="p2", bufs=2))
        pm = ctx.enter_context(tc.tile_pool(name="pm", bufs=2, space="PSUM"))
        psS = ctx.enter_context(tc.tile_pool(name="psS", bufs=2, space="PSUM"))
        psAV = ctx.enter_context(tc.tile_pool(name="psAV", bufs=1, space="PSUM"))

        # ---- resident weights ----
        W = {}
        for k, shp in WSHAPES.items():
            dt_k = DT_MM if k in MM_W else F32
            if k in SMALL_W:
                t = wp.tile(
                    [min(shp[0], 128), shp[1]], dt_k, name=f"w_{k}", tag=f"w_{k}"
                )
                nc.sync.dma_start(t[: min(shp[0], 128), :], wd[k][:, :])
                W[k] = t
            else:
                tiles = []
                for o, L in chunks(shp[0]):
                    t = wp.tile(
                        [128, shp[1]], dt_k, name=f"w_{k}_{o}", tag=f"w_{k}_{o}"
                    )
                    nc.sync.dma_start(t[:L, :], wd[k][o : o + L, :])
                    tiles.append((t, L))
                W[k] = tiles

        def alloc_row_tiles(pool, n_rows, width, tag, dtype=F32):
            out = []
            for i, (o, L) in enumerate(chunks(n_rows)):
                out.append(
                    (pool.tile([L, width], dtype, name=f"{tag}{i}", tag=f"{tag}{i}"), L)
                )
            return out

        def emit_elu(h_tiles, n_ch, neg_off, dst_tiles, width):
            """concat_elu stream: dst[c] = elu(h_c)+1, dst[neg_off+c] = elu(-h_c)+1.

            elu(x)+1 = min(exp(x),1) + relu(x); pad rows in dst (between n_ch
            and neg_off) must be pre-zeroed by the caller."""
            # + sign: exp straight into the stream (identity partitions),
            # then in-place STT; - sign: via temp E/R tiles (DVE may shift
            # partitions, ScalarE may not).
            Rp = []
            for ti, (ht, hl) in enumerate(h_tiles):
                rp = p1.tile([hl, width], F32, name=f"R{ti}", tag=f"R{ti}")
                nc.vector.tensor_scalar(rp[:hl, :], ht[:hl, :], 0.0, None, OP.max)
                Rp.append(rp)
            for st, sr, dt_, dr, L in legal_segs(0, 0, n_ch):
                d_ap = dst_tiles[dt_][0][dr : dr + L, :]
                ht = h_tiles[st][0]
                nc.scalar.activation(d_ap, ht[sr : sr + L, :], AF.Exp)
                nc.vector.scalar_tensor_tensor(
                    d_ap, d_ap, 1.0, Rp[st][sr : sr + L, :], OP.min, OP.add
                )
            EnRn = []
            for ti, (ht, hl) in enumerate(h_tiles):
                en = p1.tile([hl, width], F32, name=f"En{ti}", tag=f"En{ti}")
                nc.scalar.activation(en[:hl, :], ht[:hl, :], AF.Exp, scale=-1.0)
                rn = p1.tile([hl, width], F32, name=f"R{ti}n", tag=f"R{ti}")
                nc.vector.tensor_scalar(
                    rn[:hl, :], ht[:hl, :], 0.0, -1.0, OP.min, OP.mult
                )
                EnRn.append((en, rn))
            for st, sr, dt_, dr, L in legal_segs(0, neg_off, n_ch):
                en, rn = EnRn[st]
                nc.vector.scalar_tensor_tensor(
                    dst_tiles[dt_][0][dr : dr + L, :],
                    en[sr : sr + L, :],
                    1.0,
                    rn[sr : sr + L, :],
                    OP.min,
                    OP.add,
                )

        def emit_matmuls(psums, w_tiles, rhs_tiles, nsl):
            w_ = nsl.stop - nsl.start
            for ps, m_off, m_len in psums:
                for ki, ((wt, kl), (rt, kl2)) in enumerate(zip(w_tiles, rhs_tiles)):
                    assert kl == kl2, (kl, kl2)
                    nc.tensor.matmul(
                        ps[:m_len, 0:w_],
                        lhsT=_r(wt[:kl, m_off : m_off + m_len]),
                        rhs=_r(rt[:kl, nsl]),
                        start=(ki == 0),
                        stop=(ki == len(w_tiles) - 1),
                    )

        def _ps_segs(g_off, dst_off, length, psums):
            for st, sr, dt_, dr, L in legal_segs(
                g_off, dst_off, length, src_sbuf=False
            ):
                ps, m_off, m_len = psums[st]
                assert m_off == st * 128 and sr + L <= m_len
                yield ps, sr, dt_, dr, L, st

        def emit_gate(psums, C_, bias_t, dst_tiles, C_src, nco):
            """Gate tail of nin2: psums hold [0.5*ga | pad->192 | gb].

            T = tanh(0.5*gb + b05); U = (ga05+b05)*T; Wg = (ga05+b05)+U;
            dst = Wg + C_src   (all at one 512-col chunk)."""
            nsl = slice(nco, nco + 512)
            Tt = alloc_row_tiles(p1, C_, 512, "T")
            Ut = alloc_row_tiles(p1, C_, 512, "U")
            Wt = alloc_row_tiles(p1, C_, 512, "Wg")
            for ps, row, dt_, dr, L, m_idx in _ps_segs(PAD, 0, C_, psums):
                nc.scalar.activation(
                    Tt[dt_][0][dr : dr + L, 0:512],
                    ps[row : row + L, 0:512],
                    AF.Tanh,
                    bias=bias_t[row : row + L, m_idx : m_idx + 1],
                    scale=0.5,
                )
            for ps, row, dt_, dr, L, m_idx in _ps_segs(0, 0, C_, psums):
                b_ap = bias_t[row : row + L, m_idx : m_idx + 1]
                nc.vector.scalar_tensor_tensor(
                    Ut[dt_][0][dr : dr + L, 0:512],
                    ps[row : row + L, 0:512],
                    b_ap,
                    Tt[dt_][0][dr : dr + L, 0:512],
                    OP.add,
                    OP.mult,
                )
                nc.vector.scalar_tensor_tensor(
                    Wt[dt_][0][dr : dr + L, 0:512],
                    ps[row : row + L, 0:512],
                    b_ap,
                    Ut[dt_][0][dr : dr + L, 0:512],
                    OP.add,
                    OP.add,
                )
            for st, sr, dt_, dr, L in legal_segs(0, 0, C_):
                nc.vector.tensor_tensor(
                    dst_tiles[dt_][0][dr : dr + L, nsl],
                    Wt[st][0][sr : sr + L, 0:512],
                    C_src[st][0][sr : sr + L, nsl],
                    op=OP.add,
                )

        def zero_pad_rows(stream_tiles, n_ch):
            """memset the pad rows [n_ch, PAD) of a stream; rows n_ch-128..64
            of tile 1, via a legal window starting at 0 or 32."""
            t1 = stream_tiles[1][0]
            base = 32 if n_ch - 128 >= 32 else 0
            nc.gpsimd.memset(t1[base:64, :].bitcast(F32), 0.0)

        def inner_grn(h_tiles, C_, wo_key, bo_key, C_src, dst_tiles, width):
            """elu(h) -> nin2 -> gate -> dst = gate_out + C_src."""
            n2 = PAD + C_
            Sh = alloc_row_tiles(p1, n2, width, "Sh", DT_MM)
            zero_pad_rows(Sh, C_)
            emit_elu(h_tiles, C_, PAD, Sh, width)
            for nco in range(0, width, 512):
                psums = []
                for mi, (m_off, m_len) in enumerate(chunks(n2)):
                    ps = pm.tile(
                        [128, 512], F32, name=f"pm{mi % 2}", tag=f"pm{mi % 2}"
                    )
                    psums.append((ps, m_off, m_len))
                emit_matmuls(psums, W[wo_key], Sh, slice(nco, nco + 512))
                emit_gate(psums, C_, W[bo_key], dst_tiles, C_src, nco)

        # ---------------- per sample ----------------
        for s in range(ns):
            width = HWP

            C0 = p2.tile([128, width], F32, name="C0", tag="C0")
            C1 = p2.tile([41, width], F32, name="C1", tag="C1")
            nc.sync.dma_start(C0[:, :], ul_d[s, 0:128, :])
            nc.sync.dma_start(C1[0:32, :], ul_d[s, 128:160, :])
            nc.sync.dma_start(C1[32:38, :], b_d[s, :, :])
            nc.sync.dma_start(C1[38:41, :], x_d[s, :, :])
            C_tiles = [(C0, 128), (C1, 41)]

            Sc = alloc_row_tiles(p2, PAD + CK, width, "Sc", DT_MM)  # 361 rows
            zero_pad_rows(Sc, CK)
            emit_elu(C_tiles, CK, PAD, Sc, width)

            # fused nin1: out rows [h_k |->192| h_v |->384| h_q] = 550
            h_k = alloc_row_tiles(p1, CK, width, "hk")
            h_v = alloc_row_tiles(p1, CK, width, "hv")
            h_q = alloc_row_tiles(p1, CQ, width, "hq")
            for nco in range(0, width, 512):
                nsl = slice(nco, nco + 512)
                psums = []
                for mi, (m_off, m_len) in enumerate(chunks(550)):
                    ps = pm.tile(
                        [128, 512], F32, name=f"pm{mi % 2}", tag=f"pm{mi % 2}"
                    )
                    psums.append((ps, m_off, m_len))
                emit_matmuls(psums, W["wab_t"], Sc, nsl)
                for g_off, g_len, dsts in (
                    (0, CK, h_k),
                    (PAD, CK, h_v),
                    (2 * PAD, CQ, h_q),
                ):
                    for ps, row, dt_, dr, L, m_idx in _ps_segs(
                        g_off, 0, g_len, psums
                    ):
                        nc.scalar.activation(
                            dsts[dt_][0][dr : dr + L, nsl],
                            ps[row : row + L, 0:512],
                            AF.Identity,
                            bias=W["bab"][row : row + L, m_idx : m_idx + 1],
                        )

            # per-GRN: inner grn then its projection (keeps G lifetime short)
            K_sb = p1.tile([KD, width], DT_MM, name="Ksb", tag="Ksb")
            Q_sb = p1.tile([KD, width], DT_MM, name="Qsb", tag="Qsb")
            V_sb = p1.tile([VD, width], DT_MM, name="Vsb", tag="Vsb")
            for h_t, C_, wo_key, bo_key, wkey, dst, P_, bcol in (
                (h_k, CK, "wok_t", "bok", "nk_t", K_sb, KD, 0),
                (h_q, CQ, "woq_t", "boq", "nq_t", Q_sb, KD, 1),
                (h_v, CK, "wov_t", "bov", "nv_t", V_sb, VD, 2),
            ):
                G = alloc_row_tiles(p1, C_, width, "G", DT_MM)
                inner_grn(h_t, C_, wo_key, bo_key, C_tiles, G, width)
                for nco in range(0, width, 512):
                    nsl = slice(nco, nco + 512)
                    ps = pm.tile(
                        [128, 512], F32, name=f"pm{bcol % 2}", tag=f"pm{bcol % 2}"
                    )
                    emit_matmuls([(ps, 0, P_)], W[wkey], G, nsl)
                    nc.scalar.activation(
                        dst[:P_, nsl],
                        ps[:P_, 0:512],
                        AF.Identity,
                        bias=W["nkqvb"][:P_, bcol : bcol + 1],
                    )

            # ---- attention ----
            # S^T per 128-row k-tile [k, q], exp (no max subtraction), mask
            E_att = []
            for kt in range(8):
                h0 = kt // 4
                qstart = 512 * h0
                ew = HWP - qstart
                et = p1.tile([128, ew], DT_MM, name=f"Eatt{kt}", tag=f"Eatt{kt}")
                E_att.append((et, qstart))
                zpad = (kt % 4) * 128
                if zpad:
                    nc.gpsimd.memset(et[:, 0:zpad].bitcast(F32), 0.0)
                spans = [(128 * kt, 512 * (h0 + 1))]
                if h0 == 0:
                    spans.append((512, 1024))
                for ga, gb_ in spans:
                    ps = psS.tile([128, 512], F32, name="S", tag="S")
                    w_ = gb_ - ga
                    nc.tensor.matmul(
                        ps[:, 0:w_],
                        lhsT=_r(K_sb[:KD, kt * 128 : (kt + 1) * 128]),
                        rhs=_r(Q_sb[:KD, ga:gb_]),
                        start=True,
                        stop=True,
                    )
                    nc.scalar.activation(
                        et[:, ga - qstart : gb_ - qstart], ps[:, 0:w_], AF.Exp
                    )
                # mask the diagonal 128-block (keep q > k)
                nc.vector.tensor_tensor(
                    et[:, zpad : zpad + 128],
                    et[:, zpad : zpad + 128],
                    W["masks"][:, 0:128],
                    op=OP.mult,
                )

            # V^T (+ ones row for free softmax row sums) via PE transpose
            VT = []
            for pc in range(8):
                pst = psS.tile([128, 512], DT_MM, name="St", tag="S")
                nc.tensor.transpose(
                    pst[:, 0:80],
                    V_sb[:VD, pc * 128 : (pc + 1) * 128],
                    W["ident80"][:80, :80],
                )
                vt = p1.tile([128, 97], DT_MM, name=f"VT{pc}", tag=f"VT{pc}")
                nc.vector.tensor_copy(vt[:, 0:80], pst[:, 0:80])
                nc.gpsimd.memset(vt[:, 80:96].bitcast(F32), 0.0)
                nc.gpsimd.memset(vt[:, 96:97].bitcast(F32), 1.0)
                VT.append(vt)

            # AV accumulate over k-tiles; row 80 = sum_k E (softmax denominator)
            pav = psAV.tile([97, HWP], F32, name="AV", tag="AV")
            for qc in range(2):
                kts = [kt for kt in range(8) if 128 * kt < (qc + 1) * 512]
                for i, kt in enumerate(kts):
                    et, qstart = E_att[kt]
                    c0 = qc * 512 - qstart
                    nc.tensor.matmul(
                        pav[:97, qc * 512 : (qc + 1) * 512],
                        lhsT=_r(VT[kt][:, 0:97]),
                        rhs=_r(et[:, c0 : c0 + 512]),
                        start=(i == 0),
                        stop=(i == len(kts) - 1),
                    )

            # deferred normalization: att = AV[0:80] / ((1+eps)*R + eps*(1024-q))
            den_t = p1.tile([1, HWP], F32, name="den", tag="den")
            nc.vector.scalar_tensor_tensor(
                den_t[0:1, :],
                pav[96:97, :],
                1.0 + EPS,
                W["eps_nzq"][0:1, :],
                OP.mult,
                OP.add,
            )
            nc.vector.reciprocal_approx_fast(den_t[0:1, :], den_t[0:1, :])
            att = p1.tile([VD, HWP], F32, name="att", tag="att")
            nc.gpsimd.partition_broadcast(att[:VD, :], den_t[0:1, :])
            nc.vector.tensor_tensor(att[:VD, :], pav[0:VD, :], att[:VD, :], op=OP.mult)

            # att elu stream [Ep(80) | pad->96 | En(80)] = 176 rows
            Sa = alloc_row_tiles(p1, 176, width, "Sa", DT_MM)
            nc.gpsimd.memset(Sa[0][0][64:96, :].bitcast(F32), 0.0)  # pad rows 80:96
            emit_elu([(att, VD)], VD, 96, Sa, width)

            # ---- output GRN ----
            h_o = alloc_row_tiles(p1, NF, width, "hk")  # reuse hk tags
            w_o1 = W["wo1a_t"] + W["wo1b_t"]
            rhs_o1 = Sc + Sa
            for nco in range(0, width, 512):
                nsl = slice(nco, nco + 512)
                psums = []
                for mi, (m_off, m_len) in enumerate(chunks(NF)):
                    ps = pm.tile(
                        [128, 512], F32, name=f"pm{mi % 2}", tag=f"pm{mi % 2}"
                    )
                    psums.append((ps, m_off, m_len))
                emit_matmuls(psums, w_o1, rhs_o1, nsl)
                for ps, row, dt_, dr, L, m_idx in _ps_segs(0, 0, NF, psums):
                    nc.scalar.activation(
                        h_o[dt_][0][dr : dr + L, nsl],
                        ps[row : row + L, 0:512],
                        AF.Identity,
                        bias=W["bo1"][row : row + L, m_idx : m_idx + 1],
                    )

            O_t = alloc_row_tiles(p1, NF, width, "O")
            inner_grn(h_o, NF, "woo_t", "boo", C_tiles, O_t, width)

            nc.sync.dma_start(out_d[s, 0:128, :], O_t[0][0][:, :])
            nc.sync.dma_start(out_d[s, 128:160, :], O_t[1][0][:32, :])

    nc.compile()
    return nc


_NC_CACHE = {}


def _get_nc():
    if "nc" not in _NC_CACHE:
        _NC_CACHE["nc"] = build_nc()
    return _NC_CACHE["nc"]


def kernel(**inputs):
    inp = {
        k: np.ascontiguousarray(np.asarray(v), dtype=np.float32)
        for k, v in inputs.items()
    }
    p = prep_weights(inp)
    for k, s in WSHAPES.items():
        assert p[k].shape == s, (k, p[k].shape, s)

    x = inp["x"].reshape(N, XD, HWP)
    ul = inp["ul"].reshape(N, NF, HWP)
    b = inp["b"].reshape(N, 2 * XD, HWP)

    nc = _get_nc()
    in_maps = []
    for c in range(NCORES):
        sl = slice(c * NS, (c + 1) * NS)
        m = {"x": x[sl], "ul": ul[sl], "b": b[sl]}
        m.update(p)
        in_maps.append(m)
    res = run_bass_kernel_spmd(nc, in_maps, core_ids=list(range(NCORES)))
    out = np.concatenate([r["out"] for r in res.results], axis=0)
    return out.reshape(N, NF, 32, 32)


if __name__ == "__main__":
    import reference as R

    inputs = {k: np.asarray(v) for k, v in R.setup_inputs().items()}
    got = kernel(**inputs)
    exp = np.asarray(R.reference(**R.setup_inputs()))
    err = np.abs(got - exp)
    print("max abs err:", err.max(), "rel:", err.max() / np.abs(exp).max())



# revision 10
# speedup vs baseline: 1.0987x; 1.0987x over previous
"""Trainium2 Bass kernel for nn_AttentionBlock (causal attention block), v2.

Self-contained: takes FULL inputs (batch 32), shards batch over 8 NeuronCores
(4 samples/core, pure data parallel), runs a Bass/Tile kernel per core, and
gathers the full [32, 160, 32, 32] output.

v2 design (vs the fp32r baseline):
- bf16 matmuls and bf16 SBUF data everywhere (rel-err budget 2e-2 allows it):
  PE runs at 1 cycle/row instead of fp32r's ~3, and DVE element-wise ops get
  the 16-bit 2x mode.
- no identity-copy of nin1 outputs into fp32: h is copied PSUM->SBUF bf16 once
  (ACT identity + bias, PSUM sources may shift partitions), then all elu math
  runs 1024-wide on bf16 SBUF tiles.
- elu decomposition per sign, from m = min(h,0), rp = relu(h):
    stream_pos = exp(m) + rp        stream_neg = exp(-rp) - m
  (exp on ScalarE; min/max maps + adds on DVE/GPSIMD per ENG table).
- gate: nin2 out layout [gb | pad | 0.5*ga]; T = tanh(0.5*gb + 0.5*b_gb) + 1;
  G = (0.5*ga + 0.5*b_ga) * T.  The grn residual (+C) for the k/q/v GRNs is
  folded into the K/Q/V projection matmuls (proj(G) + proj(C)); the output
  GRN adds ul explicitly.
- K/Q/V projections run as two accumulated PSUM sets ([K|pad|Q] 48 rows, [V]
  80 rows) sharing the C-residual matmuls.
- attention identical in structure to baseline (S^T per k-tile, exp without
  max-subtraction, ones-row in V^T for free softmax denominators), in bf16.
"""

import sys

sys.path.insert(0, "/opt/trn_rl_repo")

import contextlib

import ml_dtypes
import numpy as np

import concourse.bacc as bacc
import concourse.mybir as mybir
from concourse.bass_utils import run_bass_kernel_spmd
from concourse.tile import TileContext

F32 = mybir.dt.float32
BF16 = mybir.dt.bfloat16
AF = mybir.ActivationFunctionType
OP = mybir.AluOpType
BF = ml_dtypes.bfloat16

N, XD, NF = 32, 3, 160
KD, VD = 16, 80
CK, CQ = 169, 166
HWP = 1024
NS = 4  # samples per core
NCORES = 8
EPS = 1e-7
PAD = 192  # elu- stream offset

# engine assignment knobs: 'A' = scalar/ACT, 'D' = vector/DVE, 'G' = gpsimd
ENG_MAPS = "D"   # rp/m min-max maps
ENG_TTP = "G"    # stream_pos += rp
ENG_TTN = "D"    # stream_neg = en - m
ENG_T1P = "D"    # T += 1
ENG_ORES = "G"   # output residual add


def chunks(total, step=128):
    return [(o, min(step, total - o)) for o in range(0, total, step)]


_PLIMIT = {0: 128, 32: 32, 64: 64, 96: 32}


def legal_segs(src_off, dst_off, length, src_sbuf=True):
    """Split a row-range copy into SBUF-legal pieces (windows at 0/32/64/96).
    PSUM sources are exempt.  Yields (src_tile, src_row, dst_tile, dst_row, L).
    """
    done = 0
    while done < length:
        s, d = src_off + done, dst_off + done
        sb, db = s % 128, d % 128
        L = min(_PLIMIT[db], 128 - db, length - done)
        if src_sbuf:
            L = min(L, _PLIMIT[sb], 128 - sb)
        else:
            L = min(L, 128 - sb)
        yield (s // 128, sb, d // 128, db, L)
        done += L


# ---------------------------------------------------------------- host prep --


def bias_chunked(bias):
    nm = (len(bias) + 127) // 128
    t = np.zeros((128, nm), np.float32)
    for m in range(nm):
        seg = bias[128 * m : 128 * (m + 1)]
        t[: len(seg), m] = seg
    return t


def prep_weights(inp):
    """Numpy prep: permutations, stream packing, bias folds, 0.5 gate scaling.

    Channel order 'cb' = [ul(160), b(6), x(3)].  Streams (matmul rhs rows):
    [elu+ (C) | pad->192 | elu- (C)].  nin2 out layout [gb | pad | 0.5*ga].
    Streams hold elu(x)+1, so each consumer's bias folds -W.sum(1).
    """
    p = {}
    perm_k = np.array(list(range(3, 169)) + list(range(0, 3)))
    perm_q = np.arange(166)

    def stream_cols(Wi, perm):
        C = Wi.shape[1] // 2
        W1, W2 = Wi[:, :C][:, perm], Wi[:, C:][:, perm]
        out = np.zeros((Wi.shape[0], 361), np.float32)
        out[:, : W1.shape[1]] = W1
        out[:, PAD : PAD + W2.shape[1]] = W2
        return out, Wi.sum(1)

    kW, kfold = stream_cols(inp["gkWi"], perm_k)
    vW, vfold = stream_cols(inp["gvWi"], perm_k)
    qW, qfold = stream_cols(inp["gqWi"], perm_q)
    Wab = np.zeros((550, 361), np.float32)
    Wab[0:169] = kW
    Wab[192:361] = vW
    Wab[384:550] = qW
    bab = np.zeros(550, np.float32)
    bab[0:169] = inp["gkbi"] - kfold
    bab[192:361] = inp["gvbi"] - vfold
    bab[384:550] = inp["gqbi"] - qfold
    p["wab_t"] = Wab.T  # [361, 550]
    p["bab"] = bias_chunked(bab)

    def inner_w(Wo, bo, out_perm):
        """nin2 lhsT [stream rows, out rows] with out layout [gb|pad|ga05];
        bias tile holds 0.5*b_gb at gb rows, 0.5*b_ga at ga rows."""
        C = Wo.shape[1] // 2
        W1, W2 = Wo[:, :C], Wo[:, C:]
        bias = bo - (W1.sum(1) + W2.sum(1))
        Wfull = np.concatenate([W1, W2], axis=1)  # [2C out, 2C in]
        gb_w = Wfull[C + out_perm]
        ga_w = Wfull[out_perm] * 0.5
        n = PAD + C
        Ws = np.zeros((n, n), np.float32)
        for rows, w_ in ((slice(0, C), gb_w), (slice(PAD, n), ga_w)):
            Ws[rows, 0:C] = w_[:, 0:C]
            Ws[rows, PAD : PAD + C] = w_[:, C : 2 * C]
        bs = np.zeros(n, np.float32)
        bs[0:C] = 0.5 * bias[C + out_perm]
        # ga-part bias indexed by destination row (channel), for the gate STT
        bga = np.zeros(256, np.float32)
        bga[0:C] = 0.5 * bias[out_perm]
        return Ws.T, bias_chunked(bs), bias_chunked(bga)

    p["wok_t"], p["bok"], p["bgk"] = inner_w(inp["gkWo"], inp["gkbo"], perm_k)
    p["woq_t"], p["boq"], p["bgq"] = inner_w(inp["gqWo"], inp["gqbo"], perm_q)
    p["wov_t"], p["bov"], p["bgv"] = inner_w(inp["gvWo"], inp["gvbo"], perm_k)
    p["woo_t"], p["boo"], p["bgo"] = inner_w(inp["goWo"], inp["gobo"], np.arange(NF))

    # K/Q/V projections with folded +C residual.
    nk = inp["nkW"][:, perm_k]  # [16, 169]
    nq = inp["nqW"][:, perm_q]  # [16, 166]
    nv = inp["nvW"][:, perm_k]  # [80, 169]
    pjk = np.zeros((CK, 48), np.float32)
    pjk[:, 0:16] = nk.T
    pjq = np.zeros((CQ, 48), np.float32)
    pjq[:, 32:48] = nq.T
    pjc = np.zeros((CK, 48), np.float32)
    pjc[:, 0:16] = nk.T
    pjc[0:CQ, 32:48] = nq.T
    p["pjk"], p["pjq"], p["pjc"] = pjk, pjq, pjc
    p["pjv"] = np.ascontiguousarray(nv.T)  # used for both G_v and C chunks
    njb = np.zeros((128, 2), np.float32)
    njb[0:16, 0] = inp["nkb"]
    njb[32:48, 0] = inp["nqb"]
    njb[0:80, 1] = inp["nvb"]
    p["njb"] = njb

    # grn_out nin1: Sc(ul rows) + Sa(att stream) -> h_o [160]
    W1, W2 = inp["goWi"][:, :NF], inp["goWi"][:, NF:]
    wa = np.zeros((NF, 361), np.float32)
    wa[:, 0:NF] = W1
    wa[:, PAD : PAD + NF] = W2
    p["wo1a_t"] = wa.T  # [361, 160]
    p["wo1bp"] = np.ascontiguousarray(inp["goWs"][:, :VD].T)  # [80, 160]
    p["wo1bn"] = np.ascontiguousarray(inp["goWs"][:, VD:].T)
    p["bo1"] = bias_chunked(
        (inp["gobi"] + inp["gobs"] - inp["goWi"].sum(1) - inp["goWs"].sum(1)).astype(
            np.float32
        )
    )

    pp = np.arange(128)[:, None]
    ff = np.arange(128)[None, :]
    p["masks"] = (ff > pp).astype(np.float32)
    p["eps_nzq"] = (EPS * (HWP - np.arange(HWP, dtype=np.float32)))[None, :]
    p["ident80"] = np.eye(80, dtype=np.float32)

    out = {}
    for k, v in p.items():
        dt = np.float32 if k in F32_W else BF
        out[k] = np.ascontiguousarray(np.asarray(v, dtype=np.float32).astype(dt))
    return out


WSHAPES = {
    "wab_t": (361, 550),
    "wok_t": (361, 361),
    "woq_t": (358, 358),
    "wov_t": (361, 361),
    "woo_t": (352, 352),
    "wo1a_t": (361, 160),
    "wo1bp": (80, 160),
    "wo1bn": (80, 160),
    "pjk": (169, 48),
    "pjq": (166, 48),
    "pjc": (169, 48),
    "pjv": (169, 80),
    "bab": (128, 5),
    "bok": (128, 3),
    "bgk": (128, 2),
    "bgq": (128, 2),
    "bgv": (128, 2),
    "bgo": (128, 2),
    "boq": (128, 3),
    "bov": (128, 3),
    "boo": (128, 3),
    "bo1": (128, 2),
    "njb": (128, 2),
    "masks": (128, 128),
    "eps_nzq": (1, HWP),
    "ident80": (80, 80),
}
F32_W = {"bab", "bok", "boq", "bov", "boo", "bo1", "njb", "eps_nzq", "bgk", "bgq", "bgv", "bgo"}


def build_nc(ns=NS):
    nc = bacc.Bacc("TRN2", target_bir_lowering=False, debug=False)

    x_d = nc.dram_tensor("x", [ns, XD, HWP], BF16, kind="ExternalInput")
    ul_d = nc.dram_tensor("ul", [ns, NF, HWP], BF16, kind="ExternalInput")
    b_d = nc.dram_tensor("b", [ns, 2 * XD, HWP], BF16, kind="ExternalInput")
    out_d = nc.dram_tensor("out", [ns, NF, HWP], F32, kind="ExternalOutput")
    wd = {
        k: nc.dram_tensor(k, list(s), F32 if k in F32_W else BF16, kind="ExternalInput")
        for k, s in WSHAPES.items()
    }

    with TileContext(nc) as tc, contextlib.ExitStack() as ctx:
        wp = ctx.enter_context(tc.tile_pool(name="wp", bufs=1))
        p1 = ctx.enter_context(tc.tile_pool(name="p1", bufs=1))
        p2 = ctx.enter_context(tc.tile_pool(name="p2", bufs=2))
        pm = ctx.enter_context(tc.tile_pool(name="pm", bufs=1, space="PSUM"))
        pT = ctx.enter_context(tc.tile_pool(name="pT", bufs=1, space="PSUM"))
        pAV = ctx.enter_context(tc.tile_pool(name="pAV", bufs=1, space="PSUM"))

        # ---- resident weights ----
        W = {}
        for k, shp in WSHAPES.items():
            dt_k = F32 if k in F32_W else BF16
            tiles = []
            for o, L in chunks(shp[0]):
                t = wp.tile([L, shp[1]], dt_k, name=f"w_{k}_{o}", tag=f"w_{k}_{o}")
                nc.sync.dma_start(t[:L, :], wd[k][o : o + L, :])
                tiles.append((t, L))
            W[k] = tiles

        def w1(k):
            return W[k][0][0]

        pmctr = [0]

        def pm_tile():
            i = pmctr[0] % 5
            pmctr[0] += 1
            return pm.tile([128, 512], F32, name=f"pm{i}", tag=f"pm{i}")

        def alloc_row_tiles(pool, n_rows, width, tag, dtype=BF16):
            out = []
            for i, (o, L) in enumerate(chunks(n_rows)):
                out.append(
                    (pool.tile([L, width], dtype, name=f"{tag}{i}", tag=f"{tag}{i}"), L)
                )
            return out

        def eng_ts(eng, out, in0, s1, s2, op0, op1=None):
            e = nc.vector if eng == "D" else nc.gpsimd
            if op1 is None:
                e.tensor_scalar(out, in0, s1, s2, op0)
            else:
                e.tensor_scalar(out, in0, s1, s2, op0, op1)

        def eng_tt(eng, out, a, b, op):
            e = nc.gpsimd if eng == "G" else nc.vector
            e.tensor_tensor(out, a, b, op=op)

        def emit_mm(ps_sets, pairs, nsl):
            """ps_sets: [(ps, col_off, col_len)]; pairs: [(w_tiles, rhs_tiles)]
            where w_tiles/rhs_tiles are [(tile, rows)] lists zipped per chunk."""
            w_ = nsl.stop - nsl.start
            chunk_list = []
            for w_tiles, rhs_tiles in pairs:
                for (wt, wl), (rt, rl) in zip(w_tiles, rhs_tiles):
                    assert wl == rl, (wl, rl)
                    chunk_list.append((wt, rt, wl))
            for ps, c_off, c_len in ps_sets:
                for ki, (wt, rt, kl) in enumerate(chunk_list):
                    nc.tensor.matmul(
                        ps[:c_len, 0:w_],
                        lhsT=wt[:kl, c_off : c_off + c_len],
                        rhs=rt[:kl, nsl],
                        start=(ki == 0),
                        stop=(ki == len(chunk_list) - 1),
                    )

        def _ps_segs(g_off, dst_off, length, psums):
            for st, sr, dt_, dr, L in legal_segs(g_off, dst_off, length, src_sbuf=False):
                ps, m_off, m_len = psums[st]
                assert m_off == st * 128 and sr + L <= m_len
                yield ps, sr, dt_, dr, L, st

        def copy_h(psums, g_off, C_, h_tiles, nsl, bias_t):
            """h[c, nsl] = ps[g_off+c] + bias  (ACT identity, PSUM may shift)."""
            for ps, row, dt_, dr, L, m_idx in _ps_segs(g_off, 0, C_, psums):
                nc.scalar.activation(
                    h_tiles[dt_][0][dr : dr + L, nsl],
                    ps[row : row + L, 0:512],
                    AF.Identity,
                    bias=bias_t[row : row + L, m_idx : m_idx + 1],
                )

        def emit_elu(h_tiles, C_, st_tiles, tag, width=HWP):
            """streams from bf16 h: pos = exp(m)+rp at rows [0,C); neg =
            exp(-rp)-m at rows [PAD, PAD+C).  st_tiles: 3 tiles [128,128,C-64].
            Temp tags are shared across streams (pool rotation serializes)."""
            rp = alloc_row_tiles(p2, C_, width, "rp")
            mm_ = alloc_row_tiles(p2, C_, width, "mm")
            en = alloc_row_tiles(p2, C_, width, "en")
            for i, (ht, hl) in enumerate(h_tiles):
                eng_ts(ENG_MAPS, rp[i][0][:hl, :], ht[:hl, :], 0.0, None, OP.max)
                eng_ts(ENG_MAPS, mm_[i][0][:hl, :], ht[:hl, :], 0.0, None, OP.min)
            for i, (mt, ml) in enumerate(mm_):
                st = st_tiles[i][0]
                nc.scalar.activation(st[0:ml, :], mt[:ml, :], AF.Exp)
                nc.scalar.activation(
                    en[i][0][:ml, :], rp[i][0][:ml, :], AF.Exp, scale=-1.0
                )
                eng_tt(ENG_TTP, st[0:ml, :], st[0:ml, :], rp[i][0][:ml, :], OP.add)
            for st_i, sr, dt_, dr, L in legal_segs(0, PAD, C_):
                eng_tt(
                    ENG_TTN,
                    st_tiles[dt_][0][dr : dr + L, :],
                    en[st_i][0][sr : sr + L, :],
                    mm_[st_i][0][sr : sr + L, :],
                    OP.subtract,
                )

        def emit_gate(psums, C_, bias_t, bga_t, out_tiles, nsl):
            """[gb|pad|ga05] psums -> out = (0.5ga + 0.5b_ga) * (tanh(...)+1)."""
            Tt = alloc_row_tiles(p2, C_, 512, "Tg")
            for ps, row, dt_, dr, L, m_idx in _ps_segs(0, 0, C_, psums):
                nc.scalar.activation(
                    Tt[dt_][0][dr : dr + L, 0:512],
                    ps[row : row + L, 0:512],
                    AF.Tanh,
                    bias=bias_t[row : row + L, m_idx : m_idx + 1],
                    scale=0.5,
                )
            for t_, tl in Tt:
                eng_ts(ENG_T1P, t_[:tl, 0:512], t_[:tl, 0:512], 1.0, None, OP.add)
            for ps, row, dt_, dr, L, m_idx in _ps_segs(PAD, 0, C_, psums):
                nc.vector.scalar_tensor_tensor(
                    out_tiles[dt_][0][dr : dr + L, nsl],
                    ps[row : row + L, 0:512],
                    bga_t[dr : dr + L, dt_ : dt_ + 1],
                    Tt[dt_][0][dr : dr + L, 0:512],
                    OP.add,
                    OP.mult,
                )

        # ---------------- per sample ----------------
        for s in range(ns):
            C0 = p2.tile([128, HWP], BF16, name="C0", tag="C0")
            C1 = p2.tile([41, HWP], BF16, name="C1", tag="C1")
            nc.sync.dma_start(C0[:, :], ul_d[s, 0:128, :])
            nc.sync.dma_start(C1[0:32, :], ul_d[s, 128:160, :])
            nc.sync.dma_start(C1[32:38, :], b_d[s, :, :])
            nc.sync.dma_start(C1[38:41, :], x_d[s, :, :])
            C_tiles = [(C0, 128), (C1, 41)]

            # input stream Sc [361 rows]
            Sc = alloc_row_tiles(p2, 361, HWP, "Sc")
            if s < 2:  # zero pad rows once per pool buffer (elu rewrites 32:41)
                nc.gpsimd.memset(Sc[1][0][32:64, :], 0.0)
            emit_elu(C_tiles, CK, Sc, "c")

            # fused nin1 (k,v,q): out rows [hk 0:169|pad|hv 192:361|pad|hq 384:550]
            hk = alloc_row_tiles(p2, CK, HWP, "hk")
            hv = alloc_row_tiles(p2, CK, HWP, "hv")
            hq = alloc_row_tiles(p2, CQ, HWP, "hq")
            for nco in range(0, HWP, 512):
                nsl = slice(nco, nco + 512)
                psums = []
                for m_off, m_len in chunks(550):
                    psums.append((pm_tile(), m_off, m_len))
                emit_mm(psums, [(W["wab_t"], Sc)], nsl)
                copy_h(psums, 0, CK, hk, nsl, w1("bab"))
                copy_h(psums, PAD, CK, hv, nsl, w1("bab"))
                copy_h(psums, 2 * PAD, CQ, hq, nsl, w1("bab"))

            # per-GRN: elu -> nin2 -> gate -> G (no +C; folded into projection)
            G_all = {}
            for key, h_t, C_, wo_key, bo_key, bg_key in (
                ("k", hk, CK, "wok_t", "bok", "bgk"),
                ("q", hq, CQ, "woq_t", "boq", "bgq"),
                ("v", hv, CK, "wov_t", "bov", "bgv"),
            ):
                St = alloc_row_tiles(p2, PAD + C_, HWP, f"S{key}")
                if s < 2:
                    nc.gpsimd.memset(St[1][0][32:64, :], 0.0)
                emit_elu(h_t, C_, St, key)
                G = alloc_row_tiles(p1, C_, HWP, f"G{key}")
                for nco in range(0, HWP, 512):
                    nsl = slice(nco, nco + 512)
                    psums = []
                    for m_off, m_len in chunks(PAD + C_):
                        psums.append((pm_tile(), m_off, m_len))
                    emit_mm(psums, [(W[wo_key], St)], nsl)
                    emit_gate(psums, C_, w1(bo_key), w1(bg_key), G, nsl)
                G_all[key] = G

            # K/Q/V projection (+ folded C residual)
            K_sb = p1.tile([KD, HWP], BF16, name="Ksb", tag="Ksb")
            Q_sb = p1.tile([KD, HWP], BF16, name="Qsb", tag="Qsb")
            V_sb = p1.tile([VD, HWP], BF16, name="Vsb", tag="Vsb")
            for nco in range(0, HWP, 512):
                nsl = slice(nco, nco + 512)
                ps0, ps1 = pm_tile(), pm_tile()
                emit_mm(
                    [(ps0, 0, 48)],
                    [
                        (W["pjk"], G_all["k"]),
                        (W["pjq"], G_all["q"]),
                        (W["pjc"], C_tiles),
                    ],
                    nsl,
                )
                emit_mm(
                    [(ps1, 0, 80)],
                    [(W["pjv"], G_all["v"]), (W["pjv"], C_tiles)],
                    nsl,
                )
                nc.scalar.activation(
                    K_sb[0:KD, nsl], ps0[0:KD, 0:512], AF.Identity,
                    bias=w1("njb")[0:KD, 0:1],
                )
                nc.scalar.activation(
                    Q_sb[0:KD, nsl], ps0[32:48, 0:512], AF.Identity,
                    bias=w1("njb")[32:48, 0:1],
                )
                nc.scalar.activation(
                    V_sb[0:VD, nsl], ps1[0:VD, 0:512], AF.Identity,
                    bias=w1("njb")[0:VD, 1:2],
                )

            # ---- attention ----
            E_att = []
            for kt in range(8):
                h0 = kt // 4
                qstart = 512 * h0
                ew = HWP - qstart
                et = p1.tile([128, ew], BF16, name=f"Eatt{kt}", tag=f"Eatt{kt}")
                E_att.append((et, qstart))
                zpad = (kt % 4) * 128
                if zpad:
                    nc.gpsimd.memset(et[:, 0:zpad], 0.0)
                spans = [(128 * kt, 512 * (h0 + 1))]
                if h0 == 0:
                    spans.append((512, 1024))
                for ga, gb_ in spans:
                    ps = pm_tile()
                    w_ = gb_ - ga
                    nc.tensor.matmul(
                        ps[:, 0:w_],
                        lhsT=K_sb[0:KD, kt * 128 : (kt + 1) * 128],
                        rhs=Q_sb[0:KD, ga:gb_],
                        start=True,
                        stop=True,
                    )
                    nc.scalar.activation(
                        et[:, ga - qstart : gb_ - qstart], ps[:, 0:w_], AF.Exp
                    )
                nc.vector.tensor_tensor(
                    et[:, zpad : zpad + 128],
                    et[:, zpad : zpad + 128],
                    w1("masks")[:, 0:128],
                    op=OP.mult,
                )

            # V^T (+ ones row for softmax row sums) via PE transpose
            VT = []
            for pc in range(8):
                pst = pT.tile([128, 512], BF16, name="St", tag="St")
                nc.tensor.transpose(
                    pst[:, 0:80],
                    V_sb[:VD, pc * 128 : (pc + 1) * 128],
                    w1("ident80")[:80, :80],
                )
                vt = p1.tile([128, 97], BF16, name=f"VT{pc}", tag=f"VT{pc}")
                nc.vector.tensor_copy(vt[:, 0:80], pst[:, 0:80])
                nc.gpsimd.memset(vt[:, 80:96], 0.0)
                nc.gpsimd.memset(vt[:, 96:97], 1.0)
                VT.append(vt)

            # AV accumulate; row 96 = sum_k E (softmax denominator)
            pav = pAV.tile([97, HWP], F32, name="AV", tag="AV")
            for qc in range(2):
                kts = [kt for kt in range(8) if 128 * kt < (qc + 1) * 512]
                for i, kt in enumerate(kts):
                    et, qstart = E_att[kt]
                    c0 = qc * 512 - qstart
                    nc.tensor.matmul(
                        pav[:97, qc * 512 : (qc + 1) * 512],
                        lhsT=VT[kt][:, 0:97],
                        rhs=et[:, c0 : c0 + 512],
                        start=(i == 0),
                        stop=(i == len(kts) - 1),
                    )

            # att = AV[0:80] / ((1+eps)*R + eps*(1024-q))
            den_t = p1.tile([1, HWP], F32, name="den", tag="den")
            nc.vector.scalar_tensor_tensor(
                den_t[0:1, :], pav[96:97, :], 1.0 + EPS, w1("eps_nzq")[0:1, :],
                OP.mult, OP.add,
            )
            nc.vector.reciprocal_approx_fast(den_t[0:1, :], den_t[0:1, :])
            attb = p1.tile([VD, HWP], F32, name="attb", tag="attb")
            nc.gpsimd.partition_broadcast(attb[:VD, :], den_t[0:1, :])
            att = p1.tile([VD, HWP], BF16, name="att", tag="att")
            nc.vector.tensor_tensor(att[:VD, :], pav[0:VD, :], attb[:VD, :], op=OP.mult)

            # att stream Sa: pos/neg tiles [80]
            Sa_p = p1.tile([VD, HWP], BF16, name="Sap", tag="Sap")
            Sa_n = p1.tile([VD, HWP], BF16, name="San", tag="San")
            rpa = p1.tile([VD, HWP], BF16, name="rpa", tag="rpa")
            mma = p1.tile([VD, HWP], BF16, name="mma", tag="mma")
            ena = p1.tile([VD, HWP], BF16, name="ena", tag="ena")
            eng_ts(ENG_MAPS, rpa[:VD, :], att[:VD, :], 0.0, None, OP.max)
            eng_ts(ENG_MAPS, mma[:VD, :], att[:VD, :], 0.0, None, OP.min)
            nc.scalar.activation(Sa_p[:VD, :], mma[:VD, :], AF.Exp)
            nc.scalar.activation(ena[:VD, :], rpa[:VD, :], AF.Exp, scale=-1.0)
            eng_tt(ENG_TTP, Sa_p[:VD, :], Sa_p[:VD, :], rpa[:VD, :], OP.add)
            eng_tt(ENG_TTN, Sa_n[:VD, :], ena[:VD, :], mma[:VD, :], OP.subtract)

            # ---- output GRN ----
            ho = alloc_row_tiles(p1, NF, HWP, "ho")
            for nco in range(0, HWP, 512):
                nsl = slice(nco, nco + 512)
                psums = []
                for m_off, m_len in chunks(NF):
                    psums.append((pm_tile(), m_off, m_len))
                emit_mm(
                    psums,
                    [
                        (W["wo1a_t"], Sc),
                        (W["wo1bp"], [(Sa_p, VD)]),
                        (W["wo1bn"], [(Sa_n, VD)]),
                    ],
                    nsl,
                )
                copy_h(psums, 0, NF, ho, nsl, w1("bo1"))

            So = alloc_row_tiles(p1, PAD + NF, HWP, "So")
            if s < 1:
                nc.gpsimd.memset(So[1][0][32:64, :], 0.0)
            emit_elu(ho, NF, So, "o")
            Opre = alloc_row_tiles(p1, NF, HWP, "Opre")
            O0 = p1.tile([128, HWP], F32, name="O0", tag="O0")
            O1 = p1.tile([32, HWP], F32, name="O1", tag="O1")
            for nco in range(0, HWP, 512):
                nsl = slice(nco, nco + 512)
                psums = []
                for m_off, m_len in chunks(PAD + NF):
                    psums.append((pm_tile(), m_off, m_len))
                emit_mm(psums, [(W["woo_t"], So)], nsl)
                emit_gate(psums, NF, w1("boo"), w1("bgo"), Opre, nsl)
            eng_tt(ENG_ORES, O0[:, :], Opre[0][0][:, :], C0[:, :], OP.add)
            eng_tt(ENG_ORES, O1[:32, :], Opre[1][0][:32, :], C1[0:32, :], OP.add)

            nc.sync.dma_start(out_d[s, 0:128, :], O0[:, :])
            nc.sync.dma_start(out_d[s, 128:160, :], O1[:32, :])

    nc.compile()
    return nc


_NC_CACHE = {}


def _get_nc():
    if "nc" not in _NC_CACHE:
        _NC_CACHE["nc"] = build_nc()
    return _NC_CACHE["nc"]


def make_in_maps(inputs):
    inp = {
        k: np.ascontiguousarray(np.asarray(v), dtype=np.float32)
        for k, v in inputs.items()
    }
    p = prep_weights(inp)
    for k, sshape in WSHAPES.items():
        assert p[k].shape == sshape, (k, p[k].shape, sshape)

    x = inp["x"].reshape(N, XD, HWP).astype(BF)
    ul = inp["ul"].reshape(N, NF, HWP).astype(BF)
    b = inp["b"].reshape(N, 2 * XD, HWP).astype(BF)

    in_maps = []
    for c in range(NCORES):
        sl = slice(c * NS, (c + 1) * NS)
        m = {"x": x[sl], "ul": ul[sl], "b": b[sl]}
        m.update(p)
        in_maps.append(m)
    return in_maps


def kernel(**inputs):
    in_maps = make_in_maps(inputs)
    nc = _get_nc()
    res = run_bass_kernel_spmd(nc, in_maps, core_ids=list(range(NCORES)))
    out = np.concatenate([r["out"] for r in res.results], axis=0)
    return out.reshape(N, NF, 32, 32)


if __name__ == "__main__":
    import reference as R

    inputs = {k: np.asarray(v) for k, v in R.setup_inputs().items()}
    got = kernel(**inputs)
    exp = np.asarray(R.reference(**R.setup_inputs()))
    err = np.abs(got - exp)
    print("max abs err:", err.max(), "rel:", err.max() / np.abs(exp).max())


# revision 14
# speedup vs baseline: 1.1287x; 1.0273x over previous
"""Trainium2 Bass kernel for nn_AttentionBlock (causal attention block), v2.

Self-contained: takes FULL inputs (batch 32), shards batch over 8 NeuronCores
(4 samples/core, pure data parallel), runs a Bass/Tile kernel per core, and
gathers the full [32, 160, 32, 32] output.

v2 design (vs the fp32r baseline):
- bf16 matmuls and bf16 SBUF data everywhere (rel-err budget 2e-2 allows it):
  PE runs at 1 cycle/row instead of fp32r's ~3, and DVE element-wise ops get
  the 16-bit 2x mode.
- no identity-copy of nin1 outputs into fp32: h is copied PSUM->SBUF bf16 once
  (ACT identity + bias, PSUM sources may shift partitions), then all elu math
  runs 1024-wide on bf16 SBUF tiles.
- elu decomposition per sign, from m = min(h,0), rp = relu(h):
    stream_pos = exp(m) + rp        stream_neg = exp(-rp) - m
  (exp on ScalarE; min/max maps + adds on DVE/GPSIMD per ENG table).
- gate: nin2 out layout [gb | pad | 0.5*ga]; T = tanh(0.5*gb + 0.5*b_gb) + 1;
  G = (0.5*ga + 0.5*b_ga) * T.  The grn residual (+C) for the k/q/v GRNs is
  folded into the K/Q/V projection matmuls (proj(G) + proj(C)); the output
  GRN adds ul explicitly.
- K/Q/V projections run as two accumulated PSUM sets ([K|pad|Q] 48 rows, [V]
  80 rows) sharing the C-residual matmuls.
- attention identical in structure to baseline (S^T per k-tile, exp without
  max-subtraction, ones-row in V^T for free softmax denominators), in bf16.
"""

import sys

sys.path.insert(0, "/opt/trn_rl_repo")

import contextlib

import ml_dtypes
import numpy as np

import concourse.bacc as bacc
import concourse.mybir as mybir
from concourse.bass_utils import run_bass_kernel_spmd
from concourse.tile import TileContext

F32 = mybir.dt.float32
BF16 = mybir.dt.bfloat16
AF = mybir.ActivationFunctionType
OP = mybir.AluOpType
BF = ml_dtypes.bfloat16

N, XD, NF = 32, 3, 160
KD, VD = 16, 80
CK, CQ = 169, 166
HWP = 1024
NS = 4  # samples per core
NCORES = 8
EPS = 1e-7
PAD = 192  # elu- stream offset

# engine assignment knobs: 'A' = scalar/ACT, 'D' = vector/DVE, 'G' = gpsimd
ENG_MAPS = "D"   # rp/m min-max maps
ENG_TTP = "D"    # stream_pos += rp
ENG_TTN = "D"    # stream_neg = en - m
ENG_T1P = "D"    # T += 1
ENG_ORES = "G"   # output residual add


def chunks(total, step=128):
    return [(o, min(step, total - o)) for o in range(0, total, step)]


_PLIMIT = {0: 128, 32: 32, 64: 64, 96: 32}


def legal_segs(src_off, dst_off, length, src_sbuf=True):
    """Split a row-range copy into SBUF-legal pieces (windows at 0/32/64/96).
    PSUM sources are exempt.  Yields (src_tile, src_row, dst_tile, dst_row, L).
    """
    done = 0
    while done < length:
        s, d = src_off + done, dst_off + done
        sb, db = s % 128, d % 128
        L = min(_PLIMIT[db], 128 - db, length - done)
        if src_sbuf:
            L = min(L, _PLIMIT[sb], 128 - sb)
        else:
            L = min(L, 128 - sb)
        yield (s // 128, sb, d // 128, db, L)
        done += L


# ---------------------------------------------------------------- host prep --


def bias_chunked(bias):
    nm = (len(bias) + 127) // 128
    t = np.zeros((128, nm), np.float32)
    for m in range(nm):
        seg = bias[128 * m : 128 * (m + 1)]
        t[: len(seg), m] = seg
    return t


def prep_weights(inp):
    """Numpy prep: permutations, stream packing, bias folds, 0.5 gate scaling.

    Channel order 'cb' = [ul(160), b(6), x(3)].  Streams (matmul rhs rows):
    [elu+ (C) | pad->192 | elu- (C)].  nin2 out layout [gb | pad | 0.5*ga].
    Streams hold elu(x)+1, so each consumer's bias folds -W.sum(1).
    """
    p = {}
    perm_k = np.array(list(range(3, 169)) + list(range(0, 3)))
    perm_q = np.arange(166)

    def stream_cols(Wi, perm):
        C = Wi.shape[1] // 2
        W1, W2 = Wi[:, :C][:, perm], Wi[:, C:][:, perm]
        out = np.zeros((Wi.shape[0], 361), np.float32)
        out[:, : W1.shape[1]] = W1
        out[:, PAD : PAD + W2.shape[1]] = W2
        return out, Wi.sum(1)

    kW, kfold = stream_cols(inp["gkWi"], perm_k)
    vW, vfold = stream_cols(inp["gvWi"], perm_k)
    qW, qfold = stream_cols(inp["gqWi"], perm_q)
    Wab = np.zeros((550, 361), np.float32)
    Wab[0:169] = kW
    Wab[192:361] = vW
    Wab[384:550] = qW
    bab = np.zeros(550, np.float32)
    bab[0:169] = inp["gkbi"] - kfold
    bab[192:361] = inp["gvbi"] - vfold
    bab[384:550] = inp["gqbi"] - qfold
    p["wab_t"] = Wab.T  # [361, 550]
    p["bab"] = bias_chunked(bab)

    def inner_w(Wo, bo, out_perm):
        """nin2 lhsT [stream rows, out rows] with out layout [gb|pad|ga05];
        bias tile holds 0.5*b_gb at gb rows, 0.5*b_ga at ga rows."""
        C = Wo.shape[1] // 2
        W1, W2 = Wo[:, :C], Wo[:, C:]
        bias = bo - (W1.sum(1) + W2.sum(1))
        Wfull = np.concatenate([W1, W2], axis=1)  # [2C out, 2C in]
        gb_w = Wfull[C + out_perm]
        ga_w = Wfull[out_perm] * 0.5
        n = PAD + C
        Ws = np.zeros((n, n), np.float32)
        for rows, w_ in ((slice(0, C), gb_w), (slice(PAD, n), ga_w)):
            Ws[rows, 0:C] = w_[:, 0:C]
            Ws[rows, PAD : PAD + C] = w_[:, C : 2 * C]
        bs = np.zeros(n, np.float32)
        bs[0:C] = 0.5 * bias[C + out_perm]
        # ga-part bias indexed by destination row (channel), for the gate STT
        bga = np.zeros(256, np.float32)
        bga[0:C] = 0.5 * bias[out_perm]
        return Ws.T, bias_chunked(bs), bias_chunked(bga)

    p["wok_t"], p["bok"], p["bgk"] = inner_w(inp["gkWo"], inp["gkbo"], perm_k)
    p["woq_t"], p["boq"], p["bgq"] = inner_w(inp["gqWo"], inp["gqbo"], perm_q)
    p["wov_t"], p["bov"], p["bgv"] = inner_w(inp["gvWo"], inp["gvbo"], perm_k)
    p["woo_t"], p["boo"], p["bgo"] = inner_w(inp["goWo"], inp["gobo"], np.arange(NF))

    # K/Q/V projections with folded +C residual.
    nk = inp["nkW"][:, perm_k]  # [16, 169]
    nq = inp["nqW"][:, perm_q]  # [16, 166]
    nv = inp["nvW"][:, perm_k]  # [80, 169]
    pjk = np.zeros((CK, 48), np.float32)
    pjk[:, 0:16] = nk.T
    pjq = np.zeros((CQ, 48), np.float32)
    pjq[:, 32:48] = nq.T
    pjc = np.zeros((CK, 48), np.float32)
    pjc[:, 0:16] = nk.T
    pjc[0:CQ, 32:48] = nq.T
    p["pjk"], p["pjq"], p["pjc"] = pjk, pjq, pjc
    p["pjv"] = np.ascontiguousarray(nv.T)  # used for both G_v and C chunks
    njb = np.zeros((128, 2), np.float32)
    njb[0:16, 0] = inp["nkb"]
    njb[32:48, 0] = inp["nqb"]
    njb[0:80, 1] = inp["nvb"]
    p["njb"] = njb

    # grn_out nin1: Sc(ul rows) + Sa(att stream) -> h_o [160]
    W1, W2 = inp["goWi"][:, :NF], inp["goWi"][:, NF:]
    wa = np.zeros((NF, 361), np.float32)
    wa[:, 0:NF] = W1
    wa[:, PAD : PAD + NF] = W2
    p["wo1a_t"] = wa.T  # [361, 160]
    p["wo1bp"] = np.ascontiguousarray(inp["goWs"][:, :VD].T)  # [80, 160]
    p["wo1bn"] = np.ascontiguousarray(inp["goWs"][:, VD:].T)
    p["bo1"] = bias_chunked(
        (inp["gobi"] + inp["gobs"] - inp["goWi"].sum(1) - inp["goWs"].sum(1)).astype(
            np.float32
        )
    )

    pp = np.arange(128)[:, None]
    ff = np.arange(128)[None, :]
    p["masks"] = (ff > pp).astype(np.float32)
    p["eps_nzq"] = (EPS * (HWP - np.arange(HWP, dtype=np.float32)))[None, :]
    p["ident80"] = np.eye(80, dtype=np.float32)

    out = {}
    for k, v in p.items():
        dt = np.float32 if k in F32_W else BF
        out[k] = np.ascontiguousarray(np.asarray(v, dtype=np.float32).astype(dt))
    return out


WSHAPES = {
    "wab_t": (361, 550),
    "wok_t": (361, 361),
    "woq_t": (358, 358),
    "wov_t": (361, 361),
    "woo_t": (352, 352),
    "wo1a_t": (361, 160),
    "wo1bp": (80, 160),
    "wo1bn": (80, 160),
    "pjk": (169, 48),
    "pjq": (166, 48),
    "pjc": (169, 48),
    "pjv": (169, 80),
    "bab": (128, 5),
    "bok": (128, 3),
    "bgk": (128, 2),
    "bgq": (128, 2),
    "bgv": (128, 2),
    "bgo": (128, 2),
    "boq": (128, 3),
    "bov": (128, 3),
    "boo": (128, 3),
    "bo1": (128, 2),
    "njb": (128, 2),
    "masks": (128, 128),
    "eps_nzq": (1, HWP),
    "ident80": (80, 80),
}
F32_W = {"bab", "bok", "boq", "bov", "boo", "bo1", "njb", "eps_nzq", "bgk", "bgq", "bgv", "bgo"}


def build_nc(ns=NS):
    nc = bacc.Bacc("TRN2", target_bir_lowering=False, debug=False)

    x_d = nc.dram_tensor("x", [ns, XD, HWP], BF16, kind="ExternalInput")
    ul_d = nc.dram_tensor("ul", [ns, NF, HWP], BF16, kind="ExternalInput")
    b_d = nc.dram_tensor("b", [ns, 2 * XD, HWP], BF16, kind="ExternalInput")
    out_d = nc.dram_tensor("out", [ns, NF, HWP], F32, kind="ExternalOutput")
    wd = {
        k: nc.dram_tensor(k, list(s), F32 if k in F32_W else BF16, kind="ExternalInput")
        for k, s in WSHAPES.items()
    }

    with TileContext(nc) as tc, contextlib.ExitStack() as ctx:
        wp = ctx.enter_context(tc.tile_pool(name="wp", bufs=1))
        p1 = ctx.enter_context(tc.tile_pool(name="p1", bufs=1))
        p2 = ctx.enter_context(tc.tile_pool(name="p2", bufs=2))
        pm = ctx.enter_context(tc.tile_pool(name="pm", bufs=1, space="PSUM"))
        pT = ctx.enter_context(tc.tile_pool(name="pT", bufs=1, space="PSUM"))
        pAV = ctx.enter_context(tc.tile_pool(name="pAV", bufs=1, space="PSUM"))

        # ---- resident weights ----
        W = {}
        for k, shp in WSHAPES.items():
            dt_k = F32 if k in F32_W else BF16
            tiles = []
            for o, L in chunks(shp[0]):
                t = wp.tile([L, shp[1]], dt_k, name=f"w_{k}_{o}", tag=f"w_{k}_{o}")
                nc.sync.dma_start(t[:L, :], wd[k][o : o + L, :])
                tiles.append((t, L))
            W[k] = tiles

        def w1(k):
            return W[k][0][0]

        pmctr = [0]

        def pm_tile():
            i = pmctr[0] % 5
            pmctr[0] += 1
            return pm.tile([128, 512], F32, name=f"pm{i}", tag=f"pm{i}")

        def alloc_row_tiles(pool, n_rows, width, tag, dtype=BF16):
            out = []
            for i, (o, L) in enumerate(chunks(n_rows)):
                out.append(
                    (pool.tile([L, width], dtype, name=f"{tag}{i}", tag=f"{tag}{i}"), L)
                )
            return out

        def eng_ts(eng, out, in0, s1, s2, op0, op1=None):
            e = nc.vector if eng == "D" else nc.gpsimd
            if op1 is None:
                e.tensor_scalar(out, in0, s1, s2, op0)
            else:
                e.tensor_scalar(out, in0, s1, s2, op0, op1)

        def eng_tt(eng, out, a, b, op):
            if eng == "G":
                nc.gpsimd.tensor_tensor(out, a, b, op=op)
            elif eng == "D":
                nc.vector.scalar_tensor_tensor(out, a, 0.0, b, OP.add, op)
            else:
                nc.vector.tensor_tensor(out, a, b, op=op)

        def emit_mm(ps_sets, pairs, nsl):
            """ps_sets: [(ps, col_off, col_len)]; pairs: [(w_tiles, rhs_tiles)]
            where w_tiles/rhs_tiles are [(tile, rows)] lists zipped per chunk."""
            w_ = nsl.stop - nsl.start
            chunk_list = []
            for w_tiles, rhs_tiles in pairs:
                for (wt, wl), (rt, rl) in zip(w_tiles, rhs_tiles):
                    assert wl == rl, (wl, rl)
                    chunk_list.append((wt, rt, wl))
            for ps, c_off, c_len in ps_sets:
                for ki, (wt, rt, kl) in enumerate(chunk_list):
                    nc.tensor.matmul(
                        ps[:c_len, 0:w_],
                        lhsT=wt[:kl, c_off : c_off + c_len],
                        rhs=rt[:kl, nsl],
                        start=(ki == 0),
                        stop=(ki == len(chunk_list) - 1),
                    )

        def _ps_segs(g_off, dst_off, length, psums):
            for st, sr, dt_, dr, L in legal_segs(g_off, dst_off, length, src_sbuf=False):
                ps, m_off, m_len = psums[st]
                assert m_off == st * 128 and sr + L <= m_len
                yield ps, sr, dt_, dr, L, st

        def copy_h(psums, g_off, C_, h_tiles, nsl, bias_t):
            """h[c, nsl] = ps[g_off+c] + bias  (ACT identity, PSUM may shift)."""
            for ps, row, dt_, dr, L, m_idx in _ps_segs(g_off, 0, C_, psums):
                nc.scalar.activation(
                    h_tiles[dt_][0][dr : dr + L, nsl],
                    ps[row : row + L, 0:512],
                    AF.Identity,
                    bias=bias_t[row : row + L, m_idx : m_idx + 1],
                )

        def emit_elu(h_tiles, C_, st_tiles, en_pool, en_tag, width=HWP):
            """streams from bf16 h: pos = exp(m)+rp at rows [0,C); neg =
            exp(-rp)-m at rows [PAD, PAD+C).  st_tiles: 3 tiles [128,128,C-64].
            en reuses the caller's h tags (h is dead after the rp/mm maps)."""
            rp = alloc_row_tiles(p2, C_, width, "rp")
            mm_ = alloc_row_tiles(p1, C_, width, "mm")
            en = alloc_row_tiles(en_pool, C_, width, en_tag)
            for i, (ht, hl) in enumerate(h_tiles):
                eng_ts(ENG_MAPS, rp[i][0][:hl, :], ht[:hl, :], 0.0, None, OP.max)
                eng_ts(ENG_MAPS, mm_[i][0][:hl, :], ht[:hl, :], 0.0, None, OP.min)
            for i, (mt, ml) in enumerate(mm_):
                st = st_tiles[i][0]
                nc.scalar.activation(st[0:ml, :], mt[:ml, :], AF.Exp)
                nc.scalar.activation(
                    en[i][0][:ml, :], rp[i][0][:ml, :], AF.Exp, scale=-1.0
                )
                eng_tt(ENG_TTP, st[0:ml, :], st[0:ml, :], rp[i][0][:ml, :], OP.add)
            for st_i, sr, dt_, dr, L in legal_segs(0, PAD, C_):
                eng_tt(
                    ENG_TTN,
                    st_tiles[dt_][0][dr : dr + L, :],
                    en[st_i][0][sr : sr + L, :],
                    mm_[st_i][0][sr : sr + L, :],
                    OP.subtract,
                )

        def emit_gate(psums, C_, bias_t, bga_t, out_tiles, nsl):
            """[gb|pad|ga05] psums -> out = (0.5ga + 0.5b_ga) * (tanh(...)+1)."""
            Tt = alloc_row_tiles(p1, C_, 512, "Tg")
            for ps, row, dt_, dr, L, m_idx in _ps_segs(0, 0, C_, psums):
                nc.scalar.activation(
                    Tt[dt_][0][dr : dr + L, 0:512],
                    ps[row : row + L, 0:512],
                    AF.Tanh,
                    bias=bias_t[row : row + L, m_idx : m_idx + 1],
                    scale=0.5,
                )
            for t_, tl in Tt:
                eng_ts(ENG_T1P, t_[:tl, 0:512], t_[:tl, 0:512], 1.0, None, OP.add)
            for ps, row, dt_, dr, L, m_idx in _ps_segs(PAD, 0, C_, psums):
                nc.vector.scalar_tensor_tensor(
                    out_tiles[dt_][0][dr : dr + L, nsl],
                    ps[row : row + L, 0:512],
                    bga_t[dr : dr + L, dt_ : dt_ + 1],
                    Tt[dt_][0][dr : dr + L, 0:512],
                    OP.add,
                    OP.mult,
                )

        # ---------------- per sample ----------------
        for s in range(ns):
            C0 = p2.tile([128, HWP], BF16, name="C0", tag="C0")
            C1 = p2.tile([41, HWP], BF16, name="C1", tag="C1")
            nc.sync.dma_start(C0[:, :], ul_d[s, 0:128, :])
            nc.sync.dma_start(C1[0:32, :], ul_d[s, 128:160, :])
            nc.sync.dma_start(C1[32:38, :], b_d[s, :, :])
            nc.sync.dma_start(C1[38:41, :], x_d[s, :, :])
            C_tiles = [(C0, 128), (C1, 41)]

            # input stream Sc [361 rows]
            Sc = alloc_row_tiles(p2, 361, HWP, "Sc")
            if s < 2:  # zero pad rows once per pool buffer (elu rewrites 32:41)
                nc.gpsimd.memset(Sc[1][0][32:64, :], 0.0)
            emit_elu(C_tiles, CK, Sc, p1, "enc")

            # fused nin1 (k,v,q): out rows [hk 0:169|pad|hv 192:361|pad|hq 384:550]
            hk = alloc_row_tiles(p2, CK, HWP, "hk")
            hv = alloc_row_tiles(p2, CK, HWP, "hv")
            hq = alloc_row_tiles(p2, CQ, HWP, "hq")
            for nco in range(0, HWP, 512):
                nsl = slice(nco, nco + 512)
                psums = []
                for m_off, m_len in chunks(550):
                    psums.append((pm_tile(), m_off, m_len))
                emit_mm(psums, [(W["wab_t"], Sc)], nsl)
                copy_h(psums, 0, CK, hk, nsl, w1("bab"))
                copy_h(psums, PAD, CK, hv, nsl, w1("bab"))
                copy_h(psums, 2 * PAD, CQ, hq, nsl, w1("bab"))

            # per-GRN: elu -> nin2 -> gate -> G (no +C; folded into projection)
            G_all = {}
            for key, h_t, C_, wo_key, bo_key, bg_key in (
                ("k", hk, CK, "wok_t", "bok", "bgk"),
                ("q", hq, CQ, "woq_t", "boq", "bgq"),
                ("v", hv, CK, "wov_t", "bov", "bgv"),
            ):
                St = alloc_row_tiles(p2, PAD + C_, HWP, f"S{key}")
                if s < 2:
                    nc.gpsimd.memset(St[1][0][32:64, :], 0.0)
                emit_elu(h_t, C_, St, p2, f"h{key}")
                G = alloc_row_tiles(p2, C_, HWP, f"G{key}")
                for nco in range(0, HWP, 512):
                    nsl = slice(nco, nco + 512)
                    psums = []
                    for m_off, m_len in chunks(PAD + C_):
                        psums.append((pm_tile(), m_off, m_len))
                    emit_mm(psums, [(W[wo_key], St)], nsl)
                    emit_gate(psums, C_, w1(bo_key), w1(bg_key), G, nsl)
                G_all[key] = G

            # K/Q/V projection (+ folded C residual)
            K_sb = p2.tile([KD, HWP], BF16, name="Ksb", tag="Ksb")
            Q_sb = p2.tile([KD, HWP], BF16, name="Qsb", tag="Qsb")
            V_sb = p2.tile([VD, HWP], BF16, name="Vsb", tag="Vsb")
            for nco in range(0, HWP, 512):
                nsl = slice(nco, nco + 512)
                ps0, ps1 = pm_tile(), pm_tile()
                emit_mm(
                    [(ps0, 0, 48)],
                    [
                        (W["pjk"], G_all["k"]),
                        (W["pjq"], G_all["q"]),
                        (W["pjc"], C_tiles),
                    ],
                    nsl,
                )
                emit_mm(
                    [(ps1, 0, 80)],
                    [(W["pjv"], G_all["v"]), (W["pjv"], C_tiles)],
                    nsl,
                )
                nc.scalar.activation(
                    K_sb[0:KD, nsl], ps0[0:KD, 0:512], AF.Identity,
                    bias=w1("njb")[0:KD, 0:1],
                )
                nc.scalar.activation(
                    Q_sb[0:KD, nsl], ps0[32:48, 0:512], AF.Identity,
                    bias=w1("njb")[32:48, 0:1],
                )
                nc.scalar.activation(
                    V_sb[0:VD, nsl], ps1[0:VD, 0:512], AF.Identity,
                    bias=w1("njb")[0:VD, 1:2],
                )

            # ---- attention ----
            E_att = []
            for kt in range(8):
                h0 = kt // 4
                qstart = 512 * h0
                ew = HWP - qstart
                et = p1.tile([128, ew], BF16, name=f"Eatt{kt}", tag=f"Eatt{kt}")
                E_att.append((et, qstart))
                zpad = (kt % 4) * 128
                if zpad:
                    nc.gpsimd.memset(et[:, 0:zpad], 0.0)
                spans = [(128 * kt, 512 * (h0 + 1))]
                if h0 == 0:
                    spans.append((512, 1024))
                for ga, gb_ in spans:
                    ps = pm_tile()
                    w_ = gb_ - ga
                    nc.tensor.matmul(
                        ps[:, 0:w_],
                        lhsT=K_sb[0:KD, kt * 128 : (kt + 1) * 128],
                        rhs=Q_sb[0:KD, ga:gb_],
                        start=True,
                        stop=True,
                    )
                    nc.scalar.activation(
                        et[:, ga - qstart : gb_ - qstart], ps[:, 0:w_], AF.Exp
                    )
                nc.vector.tensor_tensor(
                    et[:, zpad : zpad + 128],
                    et[:, zpad : zpad + 128],
                    w1("masks")[:, 0:128],
                    op=OP.mult,
                )

            # V^T (+ ones row for softmax row sums) via PE transpose
            VT = []
            for pc in range(8):
                pst = pT.tile([128, 512], BF16, name="St", tag="St")
                nc.tensor.transpose(
                    pst[:, 0:80],
                    V_sb[:VD, pc * 128 : (pc + 1) * 128],
                    w1("ident80")[:80, :80],
                )
                vt = p1.tile([128, 97], BF16, name=f"VT{pc}", tag=f"VT{pc}")
                nc.vector.tensor_copy(vt[:, 0:80], pst[:, 0:80])
                nc.gpsimd.memset(vt[:, 80:96], 0.0)
                nc.gpsimd.memset(vt[:, 96:97], 1.0)
                VT.append(vt)

            # AV accumulate; row 96 = sum_k E (softmax denominator)
            pav = pAV.tile([97, HWP], F32, name="AV", tag="AV")
            for qc in range(2):
                kts = [kt for kt in range(8) if 128 * kt < (qc + 1) * 512]
                for i, kt in enumerate(kts):
                    et, qstart = E_att[kt]
                    c0 = qc * 512 - qstart
                    nc.tensor.matmul(
                        pav[:97, qc * 512 : (qc + 1) * 512],
                        lhsT=VT[kt][:, 0:97],
                        rhs=et[:, c0 : c0 + 512],
                        start=(i == 0),
                        stop=(i == len(kts) - 1),
                    )

            # att = AV[0:80] / ((1+eps)*R + eps*(1024-q))
            den_t = p1.tile([1, HWP], F32, name="den", tag="den")
            nc.vector.scalar_tensor_tensor(
                den_t[0:1, :], pav[96:97, :], 1.0 + EPS, w1("eps_nzq")[0:1, :],
                OP.mult, OP.add,
            )
            nc.vector.reciprocal_approx_fast(den_t[0:1, :], den_t[0:1, :])
            attb = p1.tile([VD, HWP], F32, name="attb", tag="attb")
            nc.gpsimd.partition_broadcast(attb[:VD, :], den_t[0:1, :])
            att = p1.tile([VD, HWP], BF16, name="att", tag="att")
            nc.vector.tensor_tensor(att[:VD, :], pav[0:VD, :], attb[:VD, :], op=OP.mult)

            # att stream Sa: pos/neg tiles [80]
            Sa_p = p1.tile([VD, HWP], BF16, name="Sap", tag="Sap")
            Sa_n = p1.tile([VD, HWP], BF16, name="San", tag="San")
            rpa = p1.tile([VD, HWP], BF16, name="rpa", tag="rpa")
            mma = p1.tile([VD, HWP], BF16, name="mma", tag="mma")
            ena = p1.tile([VD, HWP], BF16, name="ena", tag="ena")
            eng_ts(ENG_MAPS, rpa[:VD, :], att[:VD, :], 0.0, None, OP.max)
            eng_ts(ENG_MAPS, mma[:VD, :], att[:VD, :], 0.0, None, OP.min)
            nc.scalar.activation(Sa_p[:VD, :], mma[:VD, :], AF.Exp)
            nc.scalar.activation(ena[:VD, :], rpa[:VD, :], AF.Exp, scale=-1.0)
            eng_tt(ENG_TTP, Sa_p[:VD, :], Sa_p[:VD, :], rpa[:VD, :], OP.add)
            eng_tt(ENG_TTN, Sa_n[:VD, :], ena[:VD, :], mma[:VD, :], OP.subtract)

            # ---- output GRN ----
            ho = alloc_row_tiles(p1, NF, HWP, "ho")
            for nco in range(0, HWP, 512):
                nsl = slice(nco, nco + 512)
                psums = []
                for m_off, m_len in chunks(NF):
                    psums.append((pm_tile(), m_off, m_len))
                emit_mm(
                    psums,
                    [
                        (W["wo1a_t"], Sc),
                        (W["wo1bp"], [(Sa_p, VD)]),
                        (W["wo1bn"], [(Sa_n, VD)]),
                    ],
                    nsl,
                )
                copy_h(psums, 0, NF, ho, nsl, w1("bo1"))

            So = alloc_row_tiles(p1, PAD + NF, HWP, "So")
            if s < 1:
                nc.gpsimd.memset(So[1][0][32:64, :], 0.0)
            emit_elu(ho, NF, So, p1, "ho")
            Opre = alloc_row_tiles(p1, NF, HWP, "Opre")
            O0 = p1.tile([128, HWP], BF16, name="O0", tag="O0")
            O1 = p1.tile([32, HWP], BF16, name="O1", tag="O1")
            for nco in range(0, HWP, 512):
                nsl = slice(nco, nco + 512)
                psums = []
                for m_off, m_len in chunks(PAD + NF):
                    psums.append((pm_tile(), m_off, m_len))
                emit_mm(psums, [(W["woo_t"], So)], nsl)
                emit_gate(psums, NF, w1("boo"), w1("bgo"), Opre, nsl)
            eng_tt(ENG_ORES, O0[:, :], Opre[0][0][:, :], C0[:, :], OP.add)
            eng_tt(ENG_ORES, O1[:32, :], Opre[1][0][:32, :], C1[0:32, :], OP.add)

            nc.gpsimd.dma_start(out_d[s, 0:128, :], O0[:, :])
            nc.gpsimd.dma_start(out_d[s, 128:160, :], O1[:32, :])

    nc.compile()
    return nc


_NC_CACHE = {}


def _get_nc():
    if "nc" not in _NC_CACHE:
        _NC_CACHE["nc"] = build_nc()
    return _NC_CACHE["nc"]


def make_in_maps(inputs):
    inp = {
        k: np.ascontiguousarray(np.asarray(v), dtype=np.float32)
        for k, v in inputs.items()
    }
    p = prep_weights(inp)
    for k, sshape in WSHAPES.items():
        assert p[k].shape == sshape, (k, p[k].shape, sshape)

    x = inp["x"].reshape(N, XD, HWP).astype(BF)
    ul = inp["ul"].reshape(N, NF, HWP).astype(BF)
    b = inp["b"].reshape(N, 2 * XD, HWP).astype(BF)

    in_maps = []
    for c in range(NCORES):
        sl = slice(c * NS, (c + 1) * NS)
        m = {"x": x[sl], "ul": ul[sl], "b": b[sl]}
        m.update(p)
        in_maps.append(m)
    return in_maps


def kernel(**inputs):
    in_maps = make_in_maps(inputs)
    nc = _get_nc()
    res = run_bass_kernel_spmd(nc, in_maps, core_ids=list(range(NCORES)))
    out = np.concatenate([r["out"] for r in res.results], axis=0)
    return out.reshape(N, NF, 32, 32)


if __name__ == "__main__":
    import reference as R

    inputs = {k: np.asarray(v) for k, v in R.setup_inputs().items()}
    got = kernel(**inputs)
    exp = np.asarray(R.reference(**R.setup_inputs()))
    err = np.abs(got - exp)
    print("max abs err:", err.max(), "rel:", err.max() / np.abs(exp).max())
